# revision 26
# baseline (speedup 1.0000x reference)
"""Causal self-attention (B=4, T=2048, C=1024, H=16) on 8 TRN2 NeuronCores.

Sharding: core c -> (batch b = c//2, head-group g = c%2 of 8 heads).
Each core computes its batch's QKV projection for its 8 heads, causal
attention, and a partial output projection (row-parallel W_proj slice).
Host sums the two partial projections per batch.

All matmuls run in fp16 (~5e-4 end-to-end relative error vs the fp32
reference); layouts avoid every transpose:
  - x is fed pre-transposed (xT [C, T]); QK^T matmuls produce Q^T/K^T
    [cols, T] directly.
  - V is produced in natural [T, cols] layout with an interleaved ones
    column per head; the PV matmul then yields O^T rows 0..63 and the
    softmax row-sums in row 64 of the same PSUM tile for free.
  - softmax skips the max subtraction (scores ~ N(0,1)), exp runs on
    [128, 1024] PSUM chunk-pair tiles, denominators from the ones
    column, normalize via fast reciprocal + partition-broadcast + mul.
  - O^T [512, T] is exactly the lhsT the output projection needs.

Scheduling: one long pipeline tuned so the PE issue stream never
starves (HAM re-throttles to half clock after ~5us PE-idle) and the
score-exp stream on ACT is spread across the span (per-head exp time
exceeds per-head score+PV matmul time, so attention needs PE filler):
  phase A : QK^T for t-chunks 0,1, V tiles 0..7, head-0 prefetch.
            Weights stream as 8 k-major [128,1536] DMAs (descriptor-
            dominated 32KB tile loads starved the old startup).
  round 0 : attention on q-chunks (0,1); filler quanta = QK^T t2,
            Q^T t3, V tiles 8..11, injected one per m2 group.
  round 1 : attention on q-chunks (2,3); filler = V 12..15, K^T t3,
            then output-projection groups for q-chunks 0,1.
  tail    : remaining projection groups (q-chunks 2,3), PE-dense.
"""

import numpy as np

B, T, C = 4, 2048, 1024
HPG, HD = 8, 64          # heads per group, head dim
GC = HPG * HD            # 512 channels per group
N_CORES = 8

_PROG = {}


def _build():
    import concourse.bacc as bacc
    import concourse.mybir as mybir
    import concourse.tile as tile

    F32 = mybir.dt.float32
    F16 = mybir.dt.float16
    EXP = mybir.ActivationFunctionType.Exp

    nc = bacc.Bacc("TRN2", target_bir_lowering=False, debug=False,
                   num_devices=N_CORES)
    xt = nc.dram_tensor("xt", [C, T], F16, kind="ExternalInput").ap()
    wqkv = nc.dram_tensor("wqkv", [C, 3 * GC], F16, kind="ExternalInput").ap()
    wp = nc.dram_tensor("wp", [GC, C], F16, kind="ExternalInput").ap()
    y = nc.dram_tensor("y", [T, C], F32, kind="ExternalOutput").ap()

    KT = C // 128       # 8 contraction tiles
    TT = T // 128       # 16 t/k tiles

    with tile.TileContext(nc) as tc:
        with (
            tc.tile_pool(name="persist", bufs=1) as persist,
            tc.tile_pool(name="ptp", bufs=17) as ptp,
            tc.tile_pool(name="rbp", bufs=4) as rbp,
            tc.tile_pool(name="rsp", bufs=4) as rsp,
            tc.tile_pool(name="ybp", bufs=6) as ybp,
            tc.tile_pool(name="acc", bufs=4, space="PSUM") as acc,
            tc.tile_pool(name="ps_s", bufs=2, space="PSUM") as ps_s,
        ):
            xt_sb = [persist.tile([128, T], F16, name=f"xt{k}", tag=f"xt{k}")
                     for k in range(KT)]

            mask = persist.tile([128, T], F16, name="mask", tag="mask")
            nc.gpsimd.memset(mask[:], 1.0)
            for d in range(4):
                nc.gpsimd.affine_select(
                    out=mask[:, 512 * d:512 * (d + 1)],
                    in_=mask[:, 512 * d:512 * (d + 1)],
                    pattern=[[1, 512]],
                    base=-128 * d,
                    channel_multiplier=-1,
                    compare_op=mybir.AluOpType.is_ge,
                    fill=0.0,
                )

            qk_sb = [persist.tile([128, T], F16, name=f"qk{c}", tag=f"qk{c}")
                     for c in range(8)]
            vext = [persist.tile([128, HPG * (HD + 1)], F16,
                                 name=f"vext{t}", tag=f"vext{t}")
                    for t in range(TT)]
            ot_sb = [persist.tile([128, T], F16, name=f"ot{i}", tag=f"ot{i}")
                     for i in range(4)]
            wp_sb = [persist.tile([128, C], F16, name=f"wp{i}", tag=f"wp{i}")
                     for i in range(4)]
            # k-major weight tiles: [Q cols | K cols | V cols] per k
            wk_sb = [persist.tile([128, 3 * GC], F16, name=f"wk{k}",
                                  tag=f"wk{k}") for k in range(KT)]

            def dma_x_half(half):
                for k in range(KT):
                    nc.sync.dma_start(
                        xt_sb[k][:, 1024 * half:1024 * (half + 1)],
                        xt[128 * k:128 * (k + 1),
                           1024 * half:1024 * (half + 1)])

            qk_ps = {}

            def emit_qk_part(c, t, ks):
                """Part of a [Q^T;K^T] col-tile accumulation (k steps in
                `ks`); filler quanta use 4-step halves (~0.85us PE)."""
                if ks[0] == 0:
                    qk_ps[(c, t)] = acc.tile([128, 512], F32,
                                             name=f"qkps{c}_{t}", tag="accps")
                pss = qk_ps[(c, t)]
                for k in ks:
                    nc.tensor.matmul(
                        pss[:], wk_sb[k][:, 128 * c:128 * (c + 1)],
                        xt_sb[k][:, 512 * t:512 * (t + 1)],
                        start=(k == 0), stop=(k == KT - 1))
                if ks[-1] == KT - 1:
                    dst = qk_sb[c][:, 512 * t:512 * (t + 1)]
                    if (c + t) % 2 and t < 2:
                        nc.scalar.copy(dst, pss[:])
                    else:
                        nc.vector.tensor_copy(dst, pss[:])
                    del qk_ps[(c, t)]

            def emit_qk_half(c, t, lohi):
                emit_qk_part(c, t, list(range(4 * lohi, 4 * lohi + 4)))

            v_ps = {}

            def emit_v_half(tt, lohi):
                """Half of a V_ext tile accumulation (ones column per
                head makes PV row-sums free)."""
                if lohi == 0:
                    v_ps[tt] = acc.tile([128, 512], F32,
                                        name=f"vps{tt}", tag="accps")
                    nc.gpsimd.memset(vext[tt].bitcast(mybir.dt.uint16),
                                     0x3C00)
                pv = v_ps[tt]
                for k in range(4 * lohi, 4 * lohi + 4):
                    nc.tensor.matmul(
                        pv[:], xt_sb[k][:, 128 * tt:128 * (tt + 1)],
                        wk_sb[k][:, 2 * GC:3 * GC],
                        start=(k == 0), stop=(k == KT - 1))
                if lohi == 1:
                    vdst = vext[tt].rearrange("p (h w) -> p h w", h=HPG)
                    nc.vector.tensor_copy(
                        vdst[:, :, 0:HD],
                        pv[:].rearrange("p (h w) -> p h w", h=HPG))
                    del v_ps[tt]

            pt_tiles = {}

            def emit_su(jp, h, m):
                """Score matmuls + exp (+ diagonal mask) for one (chunk
                pair, head, k-tile) unit -> P^T fp16 tile for PV."""
                pb = 64 * (h % 2)
                qT = qk_sb[h // 2]
                kT = qk_sb[4 + h // 2]
                d = m % 4
                jmin = m // 4
                j0, j1 = 2 * jp, 2 * jp + 1
                if jmin <= j0:
                    off = 128 * d if jmin == j0 else 0
                else:                            # only j1 valid
                    off = 512 + 128 * d
                ps = ps_s.tile([128, 1024], F32,
                               name=f"sps{jp}_{h}_{m}", tag="sps")
                for j in (j0, j1):
                    if j < jmin:
                        continue
                    o = 128 * d if j == jmin else 0
                    lo = 512 * (j - j0) + o
                    hi = 512 * (j - j0) + 512
                    nc.tensor.matmul(
                        ps[:, lo:hi],
                        kT[pb:pb + 64, 128 * m:128 * (m + 1)],
                        qT[pb:pb + 64, 512 * j + o:512 * (j + 1)],
                        start=True, stop=True)
                pt = ptp.tile([128, 1024], F16,
                              name=f"pt{jp}_{h}_{m}", tag="pt")
                nc.scalar.activation(pt[:, off:], ps[:, off:],
                                     EXP, scale=0.125)
                if jmin in (j0, j1):
                    # only the 128-wide diagonal sub-block needs masking
                    # (gpsimd tensor_mul here costs ~7us/op — DVE only)
                    mo = 512 * (jmin - j0) + 128 * d
                    eng = nc.vector
                    eng.tensor_mul(
                        pt[:, mo:mo + 128],
                        pt[:, mo:mo + 128],
                        mask[:, 512 * d + 128 * d:512 * d + 128 * d + 128])
                pt_tiles[(jp, h, m)] = pt

            def emit_proj_group(qt, n, copy_eng="vector"):
                py = acc.tile([128, 512], F32, name=f"yps{qt}_{n}",
                              tag="accps")
                for ks in range(4):
                    nc.tensor.matmul(
                        py[:],
                        ot_sb[ks][:, 128 * qt:128 * (qt + 1)],
                        wp_sb[ks][:, 512 * n:512 * (n + 1)],
                        start=(ks == 0), stop=(ks == 3))
                yb = ybp.tile([128, 512], F32, name=f"yb{qt}_{n}", tag="yb")
                if copy_eng == "scalar":
                    nc.scalar.copy(yb[:], py[:])
                else:
                    nc.vector.tensor_copy(yb[:], py[:])
                nc.sync.dma_start(
                    y[128 * qt:128 * (qt + 1), 512 * n:512 * (n + 1)], yb[:])

            def run_filler(slot):
                kind, *a = slot
                if kind == "qk":
                    emit_qk_half(*a)
                elif kind == "v":
                    emit_v_half(*a)
                elif kind == "su":
                    emit_su(*a)
                else:
                    emit_proj_group(*a)

            def attn_head(jp, h, fillers):
                """Attention for one head over chunk pair jp; fillers is
                a list of per-m2-group quantum lists (~0.85us PE each).
                Anything a unit of THIS head reads must be in a group at
                least one slot before the reading unit (Tile tracks
                dependencies backward in emission order only)."""
                j0, j1 = 2 * jp, 2 * jp + 1
                mmax = 8 * jp + 8
                po = {j: acc.tile([65, 512], F32,
                                  name=f"po{jp}_{h}_{j}", tag="accps")
                      for j in (j0, j1)}
                for mm in (0, 1):
                    if (jp, h, mm) not in pt_tiles:
                        emit_su(jp, h, mm)
                for m2 in range(0, mmax, 2):
                    # score matmuls first so the exp stream never sits
                    # behind a filler in the PE FIFO; filler before pv
                    # so same-group filler->pv reads stay emission-ordered
                    for mm in (m2 + 2, m2 + 3):
                        if mm < mmax and (jp, h, mm) not in pt_tiles:
                            emit_su(jp, h, mm)
                    if m2 == mmax - 2 and h + 1 < HPG:
                        # feed ACT across the head boundary
                        for mm in (0, 1):
                            if (jp, h + 1, mm) not in pt_tiles:
                                emit_su(jp, h + 1, mm)
                    for slot in (fillers.pop(0) if fillers else []):
                        run_filler(slot)
                    for m in (m2, m2 + 1):
                        d = m % 4
                        jmin = m // 4
                        pt = pt_tiles.pop((jp, h, m))
                        for j in (j0, j1):
                            if j < jmin:
                                continue
                            o = 128 * d if j == jmin else 0
                            nc.tensor.matmul(
                                po[j][:, o:],
                                vext[m][:, (HD + 1) * h:(HD + 1) * (h + 1)],
                                pt[:, 512 * (j - j0) + o:512 * (j - j0 + 1)],
                                start=(m == 0), stop=(m == 4 * j + 3))
                        if d == 3 and jmin in (j0, j1):
                            j = jmin
                            # NOTE: reciprocal_approx_fast with a PSUM
                            # operand NaNs on HW (sim accepts it) — the
                            # row-sum must bounce through SBUF first
                            rs = rsp.tile([1, 512], F32,
                                          name=f"rs{jp}_{h}_{j}", tag="rs")
                            nc.vector.tensor_copy(rs[:], po[j][64:65, :])
                            rc = rsp.tile([1, 512], F32,
                                          name=f"rc{jp}_{h}_{j}", tag="rc")
                            nc.vector.reciprocal_approx_fast(out=rc[:],
                                                             in_=rs[:])
                            rb = rbp.tile([64, 512], F32,
                                          name=f"rb{jp}_{h}_{j}", tag="rb")
                            nc.gpsimd.partition_broadcast(rb[:], rc[:])
                            nc.vector.tensor_mul(
                                ot_sb[h // 2][64 * (h % 2):64 * (h % 2) + 64,
                                              512 * j:512 * (j + 1)],
                                po[j][0:64, :], rb[:])

            # ---------------- phase A ---------------------------------
            for k in range(KT):
                nc.sync.dma_start(
                    xt_sb[k][:, 0:1024], xt[128 * k:128 * (k + 1), 0:1024])
                nc.sync.dma_start(wk_sb[k][:], wqkv[128 * k:128 * (k + 1), :])
            # k-outer over 4 col blocks at a time: the first matmuls need
            # only the k=0 DMA pair (640KB), so the PE chases the stream
            for k in range(KT):
                for c in (0, 4, 1, 5):
                    emit_qk_part(c, 0, [k])
            for k in range(KT):
                for c in (2, 6, 3, 7):
                    emit_qk_part(c, 0, [k])
            dma_x_half(1)
            for i in range(4):
                nc.sync.dma_start(wp_sb[i][:], wp[128 * i:128 * (i + 1), :])
            for ci, c in enumerate((0, 4, 1, 5, 2, 6, 3, 7)):
                emit_qk_half(c, 1, 0)
                emit_qk_half(c, 1, 1)
                if ci >= 4:
                    emit_su(0, 1, ci - 4)            # head-1 m 0..3
            for tt in range(TT // 2):
                emit_v_half(tt, 0)
                emit_v_half(tt, 1)
                if tt >= 1:
                    emit_su(0, 0, tt - 1)            # head-0 m 0..6
                if 2 <= tt <= 5:
                    emit_su(0, 1, tt + 2)            # head-1 m 4..7

            # ---------------- round 0: chunks (0,1) -------------------
            # one quantum per m2 group (4 groups/head); all consumers of
            # these quanta are in round 1. Heads 6/7 also prefetch the
            # first round-1 head-0 score/exp units to smooth the
            # round transition (ACT has slack here).
            f0q = []
            for c in (0, 4, 1, 5, 2, 6, 3, 7):
                f0q += [("qk", c, 2, 0), ("qk", c, 2, 1)]
            for tt in (8, 9, 10, 11):
                f0q += [("v", tt, 0), ("v", tt, 1)]
            f0 = [[[q] for q in f0q[4 * h:4 * h + 4]] for h in range(6)]
            f0 += [[[("qk", 0, 3, 0)], [("qk", 0, 3, 1)],
                    [("su", 1, 0, 0)], [("su", 1, 0, 1)]],
                   [[("qk", 1, 3, 0)], [("qk", 1, 3, 1)],
                    [("su", 1, 0, 2)], [("su", 1, 0, 3)]]]
            for h in range(HPG):
                attn_head(0, h, f0[h])

            # ---------------- round 1: chunks (2,3) -------------------
            # head 0 consumes K^T t3 (c=4) at its m=12 prefetch (m2=10)
            # and V 12..15 at pv m=12..15 — front-load those quanta
            pr = [("proj", qt, n) for qt in range(8)
                  for n in range(C // 512)]
            f1 = [
                [[("qk", 4, 3, 0), ("qk", 4, 3, 1)],
                 [("v", 12, 0), ("v", 12, 1)],
                 [("v", 13, 0)], [("v", 13, 1)], [("v", 14, 0)],
                 [("v", 14, 1)], [("v", 15, 0)], [("v", 15, 1)]],
                [[("qk", 5, 3, 0)], [("qk", 5, 3, 1)],
                 [("qk", 6, 3, 0)], [("qk", 6, 3, 1)],
                 [("qk", 7, 3, 0)], [("qk", 7, 3, 1)],
                 [("qk", 2, 3, 0)], [("qk", 2, 3, 1)]],
                [[("qk", 3, 3, 0)], [("qk", 3, 3, 1)],
                 pr[0:1], pr[1:2], [], [], [], []],
                [pr[2:3], pr[3:4], pr[4:5], [], [], [], [], []],
                [pr[5:6], pr[6:7], pr[7:8], [], [], [], [], []],
                [pr[8:9], pr[9:10], pr[10:11], [], [], [], [], []],
                [pr[11:12], pr[12:13], pr[13:14], [], [], [], [], []],
                # head 7: its chunk-2 normalize lands at the m2=10 group,
                # so chunk-2 projections fit in its last two slots
                [pr[14:15], pr[15:16], [], [], [], [],
                 [("proj", 8, 0), ("proj", 8, 1), ("proj", 9, 0)],
                 [("proj", 9, 1), ("proj", 10, 0), ("proj", 10, 1)]],
            ]
            for h in range(HPG):
                attn_head(1, h, f1[h])

            for i, (qt, n) in enumerate([(qt, n) for qt in range(11, 16)
                                         for n in range(C // 512)]):
                emit_proj_group(qt, n, "scalar" if i % 2 else "vector")

    nc.compile()
    return nc


def _get_prog():
    if "nc" not in _PROG:
        _PROG["nc"] = _build()
    return _PROG["nc"]


def make_in_maps(x, W_attn, W_proj):
    x = np.asarray(x, dtype=np.float32)
    W_attn = np.asarray(W_attn, dtype=np.float32)
    W_proj = np.asarray(W_proj, dtype=np.float32)
    f16 = np.float16
    in_maps = []
    for core in range(N_CORES):
        b, g = core // 2, core % 2
        in_maps.append({
            "xt": np.ascontiguousarray(x[b].T).astype(f16),
            "wqkv": np.ascontiguousarray(np.concatenate(
                [W_attn[:, GC * g:GC * (g + 1)],
                 W_attn[:, C + GC * g:C + GC * (g + 1)],
                 W_attn[:, 2 * C + GC * g:2 * C + GC * (g + 1)]],
                axis=1)).astype(f16),
            "wp": np.ascontiguousarray(
                W_proj[GC * g:GC * (g + 1), :]).astype(f16),
        })
    return in_maps


def run_spmd(in_maps, **kw):
    from concourse.bass_utils import run_bass_kernel_spmd
    return run_bass_kernel_spmd(_get_prog(), in_maps, list(range(N_CORES)), **kw)


def gather(results):
    out = np.empty((B, T, C), np.float32)
    for b in range(B):
        out[b] = results[2 * b]["y"] + results[2 * b + 1]["y"]
    return out


def kernel(x, W_attn, W_proj):
    res = run_spmd(make_in_maps(x, W_attn, W_proj))
    return gather(res.results)


# revision 29
# speedup vs baseline: 1.0016x; 1.0016x over previous
"""Causal self-attention (B=4, T=2048, C=1024, H=16) on 8 TRN2 NeuronCores.

Sharding: core c -> (batch b = c//2, head-group g = c%2 of 8 heads).
Each core computes its batch's QKV projection for its 8 heads, causal
attention, and a partial output projection (row-parallel W_proj slice).
Host sums the two partial projections per batch.

All matmuls run in fp16 (~5e-4 end-to-end relative error vs the fp32
reference); layouts avoid every transpose:
  - x is fed pre-transposed (xT [C, T]); QK^T matmuls produce Q^T/K^T
    [cols, T] directly.
  - V is produced in natural [T, cols] layout with an interleaved ones
    column per head; the PV matmul then yields O^T rows 0..63 and the
    softmax row-sums in row 64 of the same PSUM tile for free.
  - softmax skips the max subtraction (scores ~ N(0,1)), exp runs on
    [128, 1024] PSUM chunk-pair tiles, denominators from the ones
    column, normalize via fast reciprocal + partition-broadcast + mul.
  - O^T [512, T] is exactly the lhsT the output projection needs.

Scheduling: one long pipeline tuned so the PE issue stream never
starves (HAM re-throttles to half clock after ~5us PE-idle) and the
score-exp stream on ACT is spread across the span (per-head exp time
exceeds per-head score+PV matmul time, so attention needs PE filler):
  phase A : QK^T for t-chunks 0,1, V tiles 0..7, head-0 prefetch.
            Weights stream as 8 k-major [128,1536] DMAs (descriptor-
            dominated 32KB tile loads starved the old startup).
  round 0 : attention on q-chunks (0,1); filler quanta = QK^T t2,
            Q^T t3, V tiles 8..11, injected one per m2 group.
  round 1 : attention on q-chunks (2,3); filler = V 12..15, K^T t3,
            then output-projection groups for q-chunks 0,1.
  tail    : remaining projection groups (q-chunks 2,3), PE-dense.
"""

import numpy as np

B, T, C = 4, 2048, 1024
HPG, HD = 8, 64          # heads per group, head dim
GC = HPG * HD            # 512 channels per group
N_CORES = 8

_PROG = {}


def _build():
    import concourse.bacc as bacc
    import concourse.mybir as mybir
    import concourse.tile as tile

    F32 = mybir.dt.float32
    F16 = mybir.dt.float16
    EXP = mybir.ActivationFunctionType.Exp

    nc = bacc.Bacc("TRN2", target_bir_lowering=False, debug=False,
                   num_devices=N_CORES)
    xt = nc.dram_tensor("xt", [C, T], F16, kind="ExternalInput").ap()
    wqkv = nc.dram_tensor("wqkv", [C, 3 * GC], F16, kind="ExternalInput").ap()
    wp = nc.dram_tensor("wp", [GC, C], F16, kind="ExternalInput").ap()
    y = nc.dram_tensor("y", [T, C], F32, kind="ExternalOutput").ap()

    KT = C // 128       # 8 contraction tiles
    TT = T // 128       # 16 t/k tiles

    with tile.TileContext(nc) as tc:
        with (
            tc.tile_pool(name="persist", bufs=1) as persist,
            tc.tile_pool(name="ptp", bufs=17) as ptp,
            tc.tile_pool(name="rbp", bufs=4) as rbp,
            tc.tile_pool(name="rsp", bufs=4) as rsp,
            tc.tile_pool(name="ybp", bufs=6) as ybp,
            tc.tile_pool(name="acc", bufs=4, space="PSUM") as acc,
            tc.tile_pool(name="ps_s", bufs=2, space="PSUM") as ps_s,
        ):
            xt_sb = [persist.tile([128, T], F16, name=f"xt{k}", tag=f"xt{k}")
                     for k in range(KT)]

            mask = persist.tile([128, T], F16, name="mask", tag="mask")
            nc.gpsimd.memset(mask[:], 1.0)
            for d in range(4):
                nc.gpsimd.affine_select(
                    out=mask[:, 512 * d:512 * (d + 1)],
                    in_=mask[:, 512 * d:512 * (d + 1)],
                    pattern=[[1, 512]],
                    base=-128 * d,
                    channel_multiplier=-1,
                    compare_op=mybir.AluOpType.is_ge,
                    fill=0.0,
                )

            qk_sb = [persist.tile([128, T], F16, name=f"qk{c}", tag=f"qk{c}")
                     for c in range(8)]
            vext = [persist.tile([128, HPG * (HD + 1)], F16,
                                 name=f"vext{t}", tag=f"vext{t}")
                    for t in range(TT)]
            ot_sb = [persist.tile([128, T], F16, name=f"ot{i}", tag=f"ot{i}")
                     for i in range(4)]
            wp_sb = [persist.tile([128, C], F16, name=f"wp{i}", tag=f"wp{i}")
                     for i in range(4)]
            # k-major weight tiles: [Q cols | K cols | V cols] per k
            wk_sb = [persist.tile([128, 3 * GC], F16, name=f"wk{k}",
                                  tag=f"wk{k}") for k in range(KT)]

            def dma_x_half(half):
                for k in range(KT):
                    nc.sync.dma_start(
                        xt_sb[k][:, 1024 * half:1024 * (half + 1)],
                        xt[128 * k:128 * (k + 1),
                           1024 * half:1024 * (half + 1)])

            qk_ps = {}

            def emit_qk_part(c, t, ks):
                """Part of a [Q^T;K^T] col-tile accumulation (k steps in
                `ks`); filler quanta use 4-step halves (~0.85us PE)."""
                if ks[0] == 0:
                    qk_ps[(c, t)] = acc.tile([128, 512], F32,
                                             name=f"qkps{c}_{t}", tag="accps")
                pss = qk_ps[(c, t)]
                for k in ks:
                    nc.tensor.matmul(
                        pss[:], wk_sb[k][:, 128 * c:128 * (c + 1)],
                        xt_sb[k][:, 512 * t:512 * (t + 1)],
                        start=(k == 0), stop=(k == KT - 1))
                if ks[-1] == KT - 1:
                    dst = qk_sb[c][:, 512 * t:512 * (t + 1)]
                    if (c + t) % 2 and t < 2:
                        nc.scalar.copy(dst, pss[:])
                    else:
                        nc.vector.tensor_copy(dst, pss[:])
                    del qk_ps[(c, t)]

            def emit_qk_half(c, t, lohi):
                emit_qk_part(c, t, list(range(4 * lohi, 4 * lohi + 4)))

            v_ps = {}

            def emit_v_half(tt, lohi):
                """Half of a V_ext tile accumulation (ones column per
                head makes PV row-sums free)."""
                if lohi == 0:
                    v_ps[tt] = acc.tile([128, 512], F32,
                                        name=f"vps{tt}", tag="accps")
                    nc.gpsimd.memset(vext[tt].bitcast(mybir.dt.uint16),
                                     0x3C00)
                pv = v_ps[tt]
                for k in range(4 * lohi, 4 * lohi + 4):
                    nc.tensor.matmul(
                        pv[:], xt_sb[k][:, 128 * tt:128 * (tt + 1)],
                        wk_sb[k][:, 2 * GC:3 * GC],
                        start=(k == 0), stop=(k == KT - 1))
                if lohi == 1:
                    vdst = vext[tt].rearrange("p (h w) -> p h w", h=HPG)
                    nc.vector.tensor_copy(
                        vdst[:, :, 0:HD],
                        pv[:].rearrange("p (h w) -> p h w", h=HPG))
                    del v_ps[tt]

            pt_tiles = {}

            def emit_su(jp, h, m):
                """Score matmuls + exp (+ diagonal mask) for one (chunk
                pair, head, k-tile) unit -> P^T fp16 tile for PV."""
                pb = 64 * (h % 2)
                qT = qk_sb[h // 2]
                kT = qk_sb[4 + h // 2]
                d = m % 4
                jmin = m // 4
                j0, j1 = 2 * jp, 2 * jp + 1
                if jmin <= j0:
                    off = 128 * d if jmin == j0 else 0
                else:                            # only j1 valid
                    off = 512 + 128 * d
                ps = ps_s.tile([128, 1024], F32,
                               name=f"sps{jp}_{h}_{m}", tag="sps")
                for j in (j0, j1):
                    if j < jmin:
                        continue
                    o = 128 * d if j == jmin else 0
                    lo = 512 * (j - j0) + o
                    hi = 512 * (j - j0) + 512
                    nc.tensor.matmul(
                        ps[:, lo:hi],
                        kT[pb:pb + 64, 128 * m:128 * (m + 1)],
                        qT[pb:pb + 64, 512 * j + o:512 * (j + 1)],
                        start=True, stop=True)
                pt = ptp.tile([128, 1024], F16,
                              name=f"pt{jp}_{h}_{m}", tag="pt")
                nc.scalar.activation(pt[:, off:], ps[:, off:],
                                     EXP, scale=0.125)
                if jmin in (j0, j1):
                    # only the 128-wide diagonal sub-block needs masking
                    # (gpsimd tensor_mul here costs ~7us/op — DVE only)
                    mo = 512 * (jmin - j0) + 128 * d
                    eng = nc.vector
                    eng.tensor_mul(
                        pt[:, mo:mo + 128],
                        pt[:, mo:mo + 128],
                        mask[:, 512 * d + 128 * d:512 * d + 128 * d + 128])
                pt_tiles[(jp, h, m)] = pt

            def emit_proj_group(qt, n, copy_eng="vector"):
                py = acc.tile([128, 512], F32, name=f"yps{qt}_{n}",
                              tag="accps")
                for ks in range(4):
                    nc.tensor.matmul(
                        py[:],
                        ot_sb[ks][:, 128 * qt:128 * (qt + 1)],
                        wp_sb[ks][:, 512 * n:512 * (n + 1)],
                        start=(ks == 0), stop=(ks == 3))
                yb = ybp.tile([128, 512], F32, name=f"yb{qt}_{n}", tag="yb")
                if copy_eng == "scalar":
                    nc.scalar.copy(yb[:], py[:])
                else:
                    nc.vector.tensor_copy(yb[:], py[:])
                nc.sync.dma_start(
                    y[128 * qt:128 * (qt + 1), 512 * n:512 * (n + 1)], yb[:])

            def run_filler(slot):
                kind, *a = slot
                if kind == "qk":
                    emit_qk_half(*a)
                elif kind == "v":
                    emit_v_half(*a)
                elif kind == "su":
                    emit_su(*a)
                else:
                    emit_proj_group(*a)

            def attn_head(jp, h, fillers):
                """Attention for one head over chunk pair jp; fillers is
                a list of per-m2-group quantum lists (~0.85us PE each).
                Anything a unit of THIS head reads must be in a group at
                least one slot before the reading unit (Tile tracks
                dependencies backward in emission order only)."""
                j0, j1 = 2 * jp, 2 * jp + 1
                mmax = 8 * jp + 8
                po = {j: acc.tile([65, 512], F32,
                                  name=f"po{jp}_{h}_{j}", tag="accps")
                      for j in (j0, j1)}
                for mm in (0, 1):
                    if (jp, h, mm) not in pt_tiles:
                        emit_su(jp, h, mm)
                for m2 in range(0, mmax, 2):
                    # score matmuls first so the exp stream never sits
                    # behind a filler in the PE FIFO; filler before pv
                    # so same-group filler->pv reads stay emission-ordered
                    for mm in (m2 + 2, m2 + 3):
                        if mm < mmax and (jp, h, mm) not in pt_tiles:
                            emit_su(jp, h, mm)
                    if m2 == mmax - 2 and h + 1 < HPG:
                        # feed ACT across the head boundary
                        for mm in (0, 1):
                            if (jp, h + 1, mm) not in pt_tiles:
                                emit_su(jp, h + 1, mm)
                    for slot in (fillers.pop(0) if fillers else []):
                        run_filler(slot)
                    for m in (m2, m2 + 1):
                        d = m % 4
                        jmin = m // 4
                        pt = pt_tiles.pop((jp, h, m))
                        for j in (j0, j1):
                            if j < jmin:
                                continue
                            o = 128 * d if j == jmin else 0
                            nc.tensor.matmul(
                                po[j][:, o:],
                                vext[m][:, (HD + 1) * h:(HD + 1) * (h + 1)],
                                pt[:, 512 * (j - j0) + o:512 * (j - j0 + 1)],
                                start=(m == 0), stop=(m == 4 * j + 3))
                        if d == 3 and jmin in (j0, j1):
                            j = jmin
                            # NOTE: reciprocal_approx_fast with a PSUM
                            # operand NaNs on HW (sim accepts it) — the
                            # row-sum must bounce through SBUF first
                            rs = rsp.tile([1, 512], F32,
                                          name=f"rs{jp}_{h}_{j}", tag="rs")
                            nc.vector.tensor_copy(rs[:], po[j][64:65, :])
                            rc = rsp.tile([1, 512], F32,
                                          name=f"rc{jp}_{h}_{j}", tag="rc")
                            nc.vector.reciprocal_approx_fast(out=rc[:],
                                                             in_=rs[:])
                            rb = rbp.tile([64, 512], F32,
                                          name=f"rb{jp}_{h}_{j}", tag="rb")
                            nc.gpsimd.partition_broadcast(rb[:], rc[:])
                            nc.vector.tensor_mul(
                                ot_sb[h // 2][64 * (h % 2):64 * (h % 2) + 64,
                                              512 * j:512 * (j + 1)],
                                po[j][0:64, :], rb[:])

            # ---------------- phase A ---------------------------------
            # critical pieces first: the k-outer t0 sweep over c in
            # (0,4,1,5) needs only w cols 0:768 and x cols 0:512 per k,
            # so the first matmul is runnable after 320KB of DMA
            for k in range(KT):
                nc.sync.dma_start(wk_sb[k][:, 0:768],
                                  wqkv[128 * k:128 * (k + 1), 0:768])
                nc.sync.dma_start(
                    xt_sb[k][:, 0:512], xt[128 * k:128 * (k + 1), 0:512])
            for k in range(KT):
                nc.sync.dma_start(wk_sb[k][:, 768:1536],
                                  wqkv[128 * k:128 * (k + 1), 768:1536])
                nc.sync.dma_start(
                    xt_sb[k][:, 512:1024], xt[128 * k:128 * (k + 1), 512:1024])
            # k-outer over 4 col blocks at a time: the first matmuls need
            # only the k=0 DMA pair (640KB), so the PE chases the stream
            for k in range(KT):
                for c in (0, 4, 1, 5):
                    emit_qk_part(c, 0, [k])
            for k in range(KT):
                for c in (2, 6, 3, 7):
                    emit_qk_part(c, 0, [k])
            dma_x_half(1)
            for i in range(4):
                nc.sync.dma_start(wp_sb[i][:], wp[128 * i:128 * (i + 1), :])
            for ci, c in enumerate((0, 4, 1, 5, 2, 6, 3, 7)):
                emit_qk_half(c, 1, 0)
                emit_qk_half(c, 1, 1)
                if ci >= 4:
                    emit_su(0, 1, ci - 4)            # head-1 m 0..3
            for tt in range(TT // 2):
                emit_v_half(tt, 0)
                emit_v_half(tt, 1)
                if tt >= 1:
                    emit_su(0, 0, tt - 1)            # head-0 m 0..6
                if 2 <= tt <= 5:
                    emit_su(0, 1, tt + 2)            # head-1 m 4..7

            # ---------------- round 0: chunks (0,1) -------------------
            # one quantum per m2 group (4 groups/head); all consumers of
            # these quanta are in round 1. Heads 6/7 also prefetch the
            # first round-1 head-0 score/exp units to smooth the
            # round transition (ACT has slack here).
            f0q = []
            for c in (0, 4, 1, 5, 2, 6, 3, 7):
                f0q += [("qk", c, 2, 0), ("qk", c, 2, 1)]
            for tt in (8, 9, 10, 11):
                f0q += [("v", tt, 0), ("v", tt, 1)]
            f0 = [[[q] for q in f0q[4 * h:4 * h + 4]] for h in range(6)]
            f0 += [[[("qk", 0, 3, 0)], [("qk", 0, 3, 1)],
                    [("su", 1, 0, 0)], [("su", 1, 0, 1)]],
                   [[("qk", 1, 3, 0)], [("qk", 1, 3, 1)],
                    [("su", 1, 0, 2)], [("su", 1, 0, 3)]]]
            for h in range(HPG):
                attn_head(0, h, f0[h])

            # ---------------- round 1: chunks (2,3) -------------------
            # head 0 consumes K^T t3 (c=4) at its m=12 prefetch (m2=10)
            # and V 12..15 at pv m=12..15 — front-load those quanta
            pr = [("proj", qt, n) for qt in range(8)
                  for n in range(C // 512)]
            f1 = [
                [[("qk", 4, 3, 0), ("qk", 4, 3, 1)],
                 [("v", 12, 0), ("v", 12, 1)],
                 [("v", 13, 0)], [("v", 13, 1)], [("v", 14, 0)],
                 [("v", 14, 1)], [("v", 15, 0)], [("v", 15, 1)]],
                [[("qk", 5, 3, 0)], [("qk", 5, 3, 1)],
                 [("qk", 6, 3, 0)], [("qk", 6, 3, 1)],
                 [("qk", 7, 3, 0)], [("qk", 7, 3, 1)],
                 [("qk", 2, 3, 0)], [("qk", 2, 3, 1)]],
                [[("qk", 3, 3, 0)], [("qk", 3, 3, 1)],
                 pr[0:1], pr[1:2], [], [], [], []],
                [pr[2:3], pr[3:4], pr[4:5], [], [], [], [], []],
                [pr[5:6], pr[6:7], [], [], [], [], [], []],
                [pr[8:9], pr[9:10], [], [], [], [], [], []],
                [pr[11:12], pr[12:13], [], [], [], [], [], []],
                # head 7: its chunk-2 normalize lands at the m2=10 group,
                # so chunk-2 projections fit in its last two slots
                [pr[14:15], pr[15:16], [], [], [], [],
                 [("proj", 8, 0), ("proj", 8, 1), ("proj", 9, 0)],
                 [("proj", 9, 1), ("proj", 10, 0), ("proj", 10, 1)]],
            ]
            for h in range(HPG):
                attn_head(1, h, f1[h])

            # chunk-0/1 groups held back from round 1 run first here:
            # they have no dependency on head 7, so the PE stays dense
            # while the last normalize chains drain through DVE/GPSIMD
            for qt, n in [pr[7][1:], pr[10][1:], pr[13][1:]]:
                emit_proj_group(qt, n, "scalar")
            for qt in range(11, 16):
                for n in range(C // 512):
                    emit_proj_group(qt, n, "scalar")

    nc.compile()
    return nc


def _get_prog():
    if "nc" not in _PROG:
        _PROG["nc"] = _build()
    return _PROG["nc"]


def make_in_maps(x, W_attn, W_proj):
    x = np.asarray(x, dtype=np.float32)
    W_attn = np.asarray(W_attn, dtype=np.float32)
    W_proj = np.asarray(W_proj, dtype=np.float32)
    f16 = np.float16
    in_maps = []
    for core in range(N_CORES):
        b, g = core // 2, core % 2
        in_maps.append({
            "xt": np.ascontiguousarray(x[b].T).astype(f16),
            "wqkv": np.ascontiguousarray(np.concatenate(
                [W_attn[:, GC * g:GC * (g + 1)],
                 W_attn[:, C + GC * g:C + GC * (g + 1)],
                 W_attn[:, 2 * C + GC * g:2 * C + GC * (g + 1)]],
                axis=1)).astype(f16),
            "wp": np.ascontiguousarray(
                W_proj[GC * g:GC * (g + 1), :]).astype(f16),
        })
    return in_maps


def run_spmd(in_maps, **kw):
    from concourse.bass_utils import run_bass_kernel_spmd
    return run_bass_kernel_spmd(_get_prog(), in_maps, list(range(N_CORES)), **kw)


def gather(results):
    out = np.empty((B, T, C), np.float32)
    for b in range(B):
        out[b] = results[2 * b]["y"] + results[2 * b + 1]["y"]
    return out


def kernel(x, W_attn, W_proj):
    res = run_spmd(make_in_maps(x, W_attn, W_proj))
    return gather(res.results)
